# revision 5
# baseline (speedup 1.0000x reference)
"""Trainium2 Bass kernel for MultiInputModel (gnn_message_passing).

Math:
    gathered = state[:, idx]                       # [B, N, E]
    y   = tanh(einsum('bne,ne->bn', gathered, W) + b)   # [B, N]
    out = 500 * sigmoid(y @ Wf.T)                  # [B, A]

The gather + per-node linear is folded on the host into one dense matrix
A[c, n] = sum_e W[n, e] * [idx[n, e] == c], so the device computes two dense
matmuls with fused activations:
    yT  = tanh(A.T @ stateT + b)        # [N, Bc]  (node dim on partitions)
    out = 500 * sigmoid(yT.T @ WfT)     # [Bc, A]  (batch dim on partitions)

Sharding: batch 8192 -> 8 cores x 1024 rows; A / b / WfT replicated.
"""

import numpy as np

import concourse.bass as bass
import concourse.tile as tile
from concourse import bacc, mybir
from concourse.bass_utils import run_bass_kernel_spmd

N_CORES = 8
BATCH = 8192
B_CORE = BATCH // N_CORES  # 1024
STATE_DIM = 322
N_NODES = 256
ACTION = 4096

F32 = mybir.dt.float32
# contraction (state-dim) chunks: 322 = 128 + 128 + 66
C_CHUNKS = [(0, 128), (128, 128), (256, 66)]
AF = mybir.ActivationFunctionType


def _build_program() -> bass.Bass:
    # Bacc (not raw Bass): its compile pipeline splits multi-sem waits
    # (move_matmul_waits_to_ldweights / generate_event_semaphores) that the
    # TRN2 ISA requires — raw Bass programs fail walrus codegen on any
    # matmul with >1 semaphore wait.
    nc = bacc.Bacc("TRN2", target_bir_lowering=False, debug=False,
                   num_devices=N_CORES)

    stateT = nc.dram_tensor("stateT", [STATE_DIM, B_CORE], F32, kind="ExternalInput")
    amat = nc.dram_tensor("amat", [STATE_DIM, N_NODES], F32, kind="ExternalInput")
    bvec = nc.dram_tensor("bvec", [N_NODES, 1], F32, kind="ExternalInput")
    wfT = nc.dram_tensor("wfT", [N_NODES, ACTION], F32, kind="ExternalInput")
    out = nc.dram_tensor("out", [B_CORE, ACTION], F32, kind="ExternalOutput")

    with tile.TileContext(nc) as tc:
        with (
            tc.tile_pool(name="persist", bufs=1) as pp,
            tc.tile_pool(name="sig", bufs=4) as sigp,
            tc.tile_pool(name="obuf", bufs=3) as op,
            tc.tile_pool(name="ps_y", bufs=2, space="PSUM") as psy,
            tc.tile_pool(name="ps_o", bufs=6, space="PSUM") as pso,
        ):
            # ---- load replicated weights + this core's stateT shard ----
            s_sb, a_sb = [], []
            for ci, (c0, cl) in enumerate(C_CHUNKS):
                st = pp.tile([128, B_CORE], F32, tag=f"s{ci}")
                nc.sync.dma_start(out=st[:cl], in_=stateT[c0 : c0 + cl, :])
                s_sb.append(st)
                at = pp.tile([128, N_NODES], F32, tag=f"a{ci}")
                nc.sync.dma_start(out=at[:cl], in_=amat[c0 : c0 + cl, :])
                a_sb.append(at)

            wf_sb, b_sb, y_sb = [], [], []
            for k in range(2):
                wt = pp.tile([128, ACTION], F32, tag=f"wf{k}")
                nc.sync.dma_start(out=wt, in_=wfT[k * 128 : (k + 1) * 128, :])
                wf_sb.append(wt)
                bt = pp.tile([128, 1], F32, tag=f"b{k}")
                nc.sync.dma_start(out=bt, in_=bvec[k * 128 : (k + 1) * 128, :])
                b_sb.append(bt)
                y_sb.append(pp.tile([128, B_CORE], F32, tag=f"y{k}", name=f"y{k}"))

            # ---- phase A: yT = tanh(A.T @ stateT + b)  [256, B_CORE] ----
            for nk in range(2):
                for bj in range(B_CORE // 512):
                    ps = psy.tile([128, 512], F32)
                    for ci, (c0, cl) in enumerate(C_CHUNKS):
                        nc.tensor.matmul(
                            ps,
                            lhsT=a_sb[ci][:cl, nk * 128 : (nk + 1) * 128],
                            rhs=s_sb[ci][:cl, bj * 512 : (bj + 1) * 512],
                            start=(ci == 0),
                            stop=(ci == len(C_CHUNKS) - 1),
                        )
                    nc.scalar.activation(
                        out=y_sb[nk][:, bj * 512 : (bj + 1) * 512],
                        in_=ps,
                        func=AF.Tanh,
                        bias=b_sb[nk],
                        scale=1.0,
                    )

            # ---- phase B: out = 500 * sigmoid(yT.T @ WfT)  [B_CORE, A] ----
            for bi in range(B_CORE // 128):
                ot = op.tile([128, ACTION], F32, tag="ot")
                for ai in range(ACTION // 512):
                    ps = pso.tile([128, 512], F32)
                    for k in range(2):
                        nc.tensor.matmul(
                            ps,
                            lhsT=y_sb[k][:, bi * 128 : (bi + 1) * 128],
                            rhs=wf_sb[k][:, ai * 512 : (ai + 1) * 512],
                            start=(k == 0),
                            stop=(k == 1),
                        )
                    sg = sigp.tile([128, 512], F32, tag="sg")
                    nc.scalar.activation(out=sg, in_=ps, func=AF.Sigmoid)
                    nc.vector.tensor_scalar_mul(
                        ot[:, ai * 512 : (ai + 1) * 512], sg, 500.0
                    )
                nc.sync.dma_start(out=out[bi * 128 : (bi + 1) * 128, :], in_=ot)

    nc.finalize()  # Bacc.finalize -> compile(): reg alloc, wait splitting, ...
    return nc


def _prepare_in_maps(state, W, b, Wf, idx):
    state = np.asarray(state, dtype=np.float32)
    W = np.asarray(W, dtype=np.float32)
    b = np.asarray(b, dtype=np.float32)
    Wf = np.asarray(Wf, dtype=np.float32)
    idx = np.asarray(idx)

    # Fold gather+per-node-linear into one dense [STATE_DIM, N_NODES] matrix.
    amat = np.zeros((STATE_DIM, N_NODES), dtype=np.float32)
    cols = np.broadcast_to(np.arange(N_NODES, dtype=np.int64)[:, None], idx.shape)
    np.add.at(amat, (idx.astype(np.int64), cols), W)

    stateT = np.ascontiguousarray(state.T)  # [STATE_DIM, BATCH]
    wfT = np.ascontiguousarray(Wf.T)  # [N_NODES, ACTION]
    bcol = np.ascontiguousarray(b.reshape(N_NODES, 1))

    in_maps = []
    for i in range(N_CORES):
        in_maps.append(
            {
                "stateT": np.ascontiguousarray(
                    stateT[:, i * B_CORE : (i + 1) * B_CORE]
                ),
                "amat": amat,
                "bvec": bcol,
                "wfT": wfT,
            }
        )
    return in_maps


def _run(inputs: dict, trace: bool = False):
    nc = _build_program()
    in_maps = _prepare_in_maps(**inputs)
    res = run_bass_kernel_spmd(
        nc, in_maps, list(range(N_CORES)), trace=trace
    )
    out = np.concatenate([res.results[i]["out"] for i in range(N_CORES)], axis=0)
    return out, res


def kernel(**inputs) -> np.ndarray:
    out, _ = _run(inputs, trace=False)
    return out


if __name__ == "__main__":
    rng = np.random.default_rng(0)
    demo = {
        "state": rng.standard_normal((BATCH, STATE_DIM), dtype=np.float32),
        "W": rng.standard_normal((N_NODES, 27), dtype=np.float32),
        "b": rng.standard_normal(N_NODES, dtype=np.float32),
        "Wf": rng.standard_normal((ACTION, N_NODES), dtype=np.float32),
        "idx": rng.integers(0, STATE_DIM, size=(N_NODES, 27)).astype(np.int32),
    }
    o = kernel(**demo)
    print(o.shape, o.dtype)


# revision 10
# speedup vs baseline: 1.7990x; 1.7990x over previous
"""Trainium2 Bass kernel for MultiInputModel (gnn_message_passing).

Math:
    gathered = state[:, idx]                       # [B, N, E]
    y   = tanh(einsum('bne,ne->bn', gathered, W) + b)   # [B, N]
    out = 500 * sigmoid(y @ Wf.T)                  # [B, A]

The gather + per-node linear is folded on the host into one dense matrix
A[c, n] = sum_e W[n, e] * [idx[n, e] == c], so the device computes two dense
matmuls with fused activations:
    yT  = tanh(A.T @ stateT + b)        # [N, Bc]  (node dim on partitions)
    out = 500 * sigmoid(yT.T @ WfT)     # [Bc, A]  (batch dim on partitions)

Sharding: batch 8192 -> 8 cores x 1024 rows; A / b / WfT replicated.
"""

import numpy as np

import concourse.bass as bass
import concourse.tile as tile
from concourse import bacc, mybir
from concourse.bass_utils import run_bass_kernel_spmd

N_CORES = 8
BATCH = 8192
B_CORE = BATCH // N_CORES  # 1024
STATE_DIM = 322
N_NODES = 256
ACTION = 4096

F32 = mybir.dt.float32
F32R = mybir.dt.float32r  # single-pass PE matmul (1 cyc/row vs 4 for fp32)
# contraction (state-dim) chunks: 322 = 128 + 128 + 66
C_CHUNKS = [(0, 128), (128, 128), (256, 66)]
AF = mybir.ActivationFunctionType


def _build_program() -> bass.Bass:
    # Bacc (not raw Bass): its compile pipeline splits multi-sem waits
    # (move_matmul_waits_to_ldweights / generate_event_semaphores) that the
    # TRN2 ISA requires — raw Bass programs fail walrus codegen on any
    # matmul with >1 semaphore wait.
    nc = bacc.Bacc("TRN2", target_bir_lowering=False, debug=False,
                   num_devices=N_CORES)

    stateT = nc.dram_tensor("stateT", [STATE_DIM, B_CORE], F32R, kind="ExternalInput")
    amat = nc.dram_tensor("amat", [STATE_DIM, N_NODES], F32R, kind="ExternalInput")
    bvec = nc.dram_tensor("bvec", [N_NODES, 1], F32, kind="ExternalInput")
    wfT = nc.dram_tensor("wfT", [N_NODES, ACTION], F32R, kind="ExternalInput")
    out = nc.dram_tensor("out", [B_CORE, ACTION], F32, kind="ExternalOutput")

    with tile.TileContext(nc) as tc:
        with (
            tc.tile_pool(name="persist", bufs=1) as pp,
            tc.tile_pool(name="sig", bufs=4) as sigp,
            tc.tile_pool(name="obuf", bufs=3) as op,
            tc.tile_pool(name="ps_y", bufs=2, space="PSUM") as psy,
            tc.tile_pool(name="ps_o", bufs=6, space="PSUM") as pso,
        ):
            # ---- load replicated weights + this core's stateT shard ----
            s_sb, a_sb = [], []
            for ci, (c0, cl) in enumerate(C_CHUNKS):
                st = pp.tile([128, B_CORE], F32R, tag=f"s{ci}")
                nc.sync.dma_start(out=st[:cl], in_=stateT[c0 : c0 + cl, :])
                s_sb.append(st)
                at = pp.tile([128, N_NODES], F32R, tag=f"a{ci}")
                nc.sync.dma_start(out=at[:cl], in_=amat[c0 : c0 + cl, :])
                a_sb.append(at)

            wf_sb, b_sb, y_sb = [], [], []
            for k in range(2):
                wt = pp.tile([128, ACTION], F32R, tag=f"wf{k}")
                nc.sync.dma_start(out=wt, in_=wfT[k * 128 : (k + 1) * 128, :])
                wf_sb.append(wt)
                bt = pp.tile([128, 1], F32, tag=f"b{k}")
                nc.sync.dma_start(out=bt, in_=bvec[k * 128 : (k + 1) * 128, :])
                b_sb.append(bt)
                y_sb.append(pp.tile([128, B_CORE], F32R, tag=f"y{k}", name=f"y{k}"))

            # ---- phase A: yT = tanh(A.T @ stateT + b)  [256, B_CORE] ----
            for nk in range(2):
                for bj in range(B_CORE // 512):
                    ps = psy.tile([128, 512], F32)
                    for ci, (c0, cl) in enumerate(C_CHUNKS):
                        nc.tensor.matmul(
                            ps,
                            lhsT=a_sb[ci][:cl, nk * 128 : (nk + 1) * 128],
                            rhs=s_sb[ci][:cl, bj * 512 : (bj + 1) * 512],
                            start=(ci == 0),
                            stop=(ci == len(C_CHUNKS) - 1),
                        )
                    nc.scalar.activation(
                        out=y_sb[nk][:, bj * 512 : (bj + 1) * 512],
                        in_=ps,
                        func=AF.Tanh,
                        bias=b_sb[nk],
                        scale=1.0,
                    )

            # ---- phase B: out = 500 * sigmoid(yT.T @ WfT)  [B_CORE, A] ----
            for bi in range(B_CORE // 128):
                ot = op.tile([128, ACTION], F32, tag="ot")
                for ai in range(ACTION // 512):
                    ps = pso.tile([128, 512], F32)
                    for k in range(2):
                        nc.tensor.matmul(
                            ps,
                            lhsT=y_sb[k][:, bi * 128 : (bi + 1) * 128],
                            rhs=wf_sb[k][:, ai * 512 : (ai + 1) * 512],
                            start=(k == 0),
                            stop=(k == 1),
                        )
                    sg = sigp.tile([128, 512], F32, tag="sg")
                    nc.scalar.activation(out=sg, in_=ps, func=AF.Sigmoid)
                    nc.vector.tensor_scalar_mul(
                        ot[:, ai * 512 : (ai + 1) * 512], sg, 500.0
                    )
                nc.sync.dma_start(out=out[bi * 128 : (bi + 1) * 128, :], in_=ot)

    nc.finalize()  # Bacc.finalize -> compile(): reg alloc, wait splitting, ...
    return nc


def _prepare_in_maps(state, W, b, Wf, idx):
    state = np.asarray(state, dtype=np.float32)
    W = np.asarray(W, dtype=np.float32)
    b = np.asarray(b, dtype=np.float32)
    Wf = np.asarray(Wf, dtype=np.float32)
    idx = np.asarray(idx)

    # Fold gather+per-node-linear into one dense [STATE_DIM, N_NODES] matrix.
    amat = np.zeros((STATE_DIM, N_NODES), dtype=np.float32)
    cols = np.broadcast_to(np.arange(N_NODES, dtype=np.int64)[:, None], idx.shape)
    np.add.at(amat, (idx.astype(np.int64), cols), W)

    stateT = np.ascontiguousarray(state.T)  # [STATE_DIM, BATCH]
    wfT = np.ascontiguousarray(Wf.T)  # [N_NODES, ACTION]
    bcol = np.ascontiguousarray(b.reshape(N_NODES, 1))

    in_maps = []
    for i in range(N_CORES):
        in_maps.append(
            {
                "stateT": np.ascontiguousarray(
                    stateT[:, i * B_CORE : (i + 1) * B_CORE]
                ),
                "amat": amat,
                "bvec": bcol,
                "wfT": wfT,
            }
        )
    return in_maps


def _run(inputs: dict, trace: bool = False):
    nc = _build_program()
    in_maps = _prepare_in_maps(**inputs)
    res = run_bass_kernel_spmd(
        nc, in_maps, list(range(N_CORES)), trace=trace
    )
    out = np.concatenate([res.results[i]["out"] for i in range(N_CORES)], axis=0)
    return out, res


def kernel(**inputs) -> np.ndarray:
    out, _ = _run(inputs, trace=False)
    return out


if __name__ == "__main__":
    rng = np.random.default_rng(0)
    demo = {
        "state": rng.standard_normal((BATCH, STATE_DIM), dtype=np.float32),
        "W": rng.standard_normal((N_NODES, 27), dtype=np.float32),
        "b": rng.standard_normal(N_NODES, dtype=np.float32),
        "Wf": rng.standard_normal((ACTION, N_NODES), dtype=np.float32),
        "idx": rng.integers(0, STATE_DIM, size=(N_NODES, 27)).astype(np.int32),
    }
    o = kernel(**demo)
    print(o.shape, o.dtype)
